# revision 1
# baseline (speedup 1.0000x reference)
"""CosineSimilarityAttention Trainium2 kernel (8 NeuronCores, SPMD).

Sharding: token-parallel. Global tokens = 2 batches x 4096. Core c handles
batch (c // 4), query rows (c % 4)*1024 .. +1024. Each core computes K/V
projections for its whole batch (4096 tokens) -- replicated within each
4-core batch group -- plus Q for its own 1024 tokens, then attention and
the output projection for its token slice. Outputs concatenate on host.

Math per batch (faithful to reference):
  qkv = x @ w_qkv.T ; split q,k,v ; reshape heads h=12, dh=64
  q *= 1/sqrt(||q||_heads + eps)   (L2 norm over the HEADS axis, per (n, dh))
  k *= 1/sqrt(||k||_heads + eps)
  out_h = softmax((q_h k_h^T) / scale_h) v_h   (no max-subtract: |logits|<~2)
  y = concat_h(out_h) @ w_out.T + b_out
"""

import numpy as np

import concourse.bass as bass
import concourse.mybir as mybir
import concourse.tile as tile
from concourse.bass_utils import run_bass_kernel_spmd
from concourse.masks import make_identity

F32 = mybir.dt.float32
BF16 = mybir.dt.bfloat16

B = 2
N = 4096          # tokens per batch
D = 768           # model dim
H = 12            # heads
DH = 64           # head dim
INNER = H * DH    # 768
EPS = 1e-8
NQ = 1024         # query tokens per core
NCORES = 8
BLK = 512         # projection token block
KT = N // 128     # 32 key tiles of 128


def _split_multi_waits(nc):
    """This container's walrus accepts only ONE sync-wait per instruction.
    Hoist extra waits into standalone EVSEM instructions placed just before."""
    n = 0
    for f in nc.m.functions:
        for bb in f.blocks:
            insts = list(bb.instructions)
            out = []
            for inst in insts:
                si = inst.sync_info
                if si is not None and si.on_wait is not None and len(si.on_wait) > 1:
                    waits = list(si.on_wait)
                    for j, w in enumerate(waits[:-1]):
                        ev = mybir.InstEventSemaphore(
                            name=f"{inst.name}-evw{j}",
                            engine=inst.engine,
                            sync_info=mybir.SyncInfo(on_wait=[w], on_update=[]),
                        )
                        out.append(ev)
                        n += 1
                    si.on_wait = [waits[-1]]
                out.append(inst)
            bb.instructions = out
    return n


def _build_program(inv_scale):
    """Build the single SPMD Bass program. inv_scale: list of 12 floats."""
    nc = bass.Bass()
    xb = nc.declare_dram_parameter("xb", [N, D], F32, isOutput=False)
    qx = nc.declare_dram_parameter("qx", [NQ, D], F32, isOutput=False)
    wqkvT = nc.declare_dram_parameter("wqkvT", [D, 3 * INNER], F32, isOutput=False)
    woT = nc.declare_dram_parameter("woT", [INNER, D], F32, isOutput=False)
    bout = nc.declare_dram_parameter("bout", [1, D], F32, isOutput=False)
    selin = nc.declare_dram_parameter("selin", [128, 128], F32, isOutput=False)
    y = nc.declare_dram_parameter("y", [NQ, D], F32, isOutput=True)

    with tile.TileContext(nc) as tc:
        with tc.tile_pool(name="const", bufs=1) as constp, \
             tc.tile_pool(name="persist", bufs=1) as persist:
            # --- constants ---
            ident = constp.tile([128, 128], F32)
            make_identity(nc, ident)
            sel_st = constp.tile([128, 128], F32)
            nc.sync.dma_start(out=sel_st, in_=selin[:, :])
            sel_bf = constp.tile([128, 128], BF16)
            nc.vector.tensor_copy(sel_bf, sel_st)
            ones_f = constp.tile([1, 64], F32)
            nc.vector.memset(ones_f, 1.0)
            ones_bf = constp.tile([1, 128], BF16)
            nc.vector.memset(ones_bf, 1.0)
            eps_t = constp.tile([128, 1], F32)
            nc.vector.memset(eps_t, EPS)
            invs = constp.tile([128, 6], F32)
            for dt in range(6):
                nc.vector.memset(invs[0:64, dt:dt + 1], float(inv_scale[2 * dt]))
                nc.vector.memset(invs[64:128, dt:dt + 1], float(inv_scale[2 * dt + 1]))
            b_st = constp.tile([1, D], F32)
            nc.sync.dma_start(out=b_st, in_=bout[:, :])
            b_bf = constp.tile([1, D], BF16)
            nc.vector.tensor_copy(b_bf, b_st)

            # --- persistent activations ---
            khat = persist.tile([128, 6, N], BF16)     # k^T normalized  [dim, tok]
            qhat = persist.tile([128, H, NQ], BF16)    # q^T per head, K=128 zero-padded
            vhat = persist.tile([128, KT, H * 65], BF16)  # v [tok, h*65] (+ones col)

            # ones columns of vhat (col 64 of every 65-block)
            vones = vhat.rearrange("p t (h c) -> p t h c", c=65)[:, :, :, 64:65]
            nc.vector.memset(vones, 1.0)
            nc.vector.memset(qhat, 0.0)

            # ---------------- phase W+P: weights, projections, head-norm ----
            with tc.tile_pool(name="pw", bufs=1) as pwp:
              wq = pwp.tile([128, 6, 3 * INNER], BF16)
              with tc.tile_pool(name="wstage", bufs=1) as wst:
                for dt in range(6):
                    st = wst.tile([128, 3 * INNER], F32, tag="wst")
                    nc.sync.dma_start(out=st, in_=wqkvT[dt * 128:(dt + 1) * 128, :])
                    nc.vector.tensor_copy(wq[:, dt, :], st)
              with tc.tile_pool(name="pstage", bufs=1) as pstage, \
                   tc.tile_pool(name="pxT", bufs=2) as pxT, \
                   tc.tile_pool(name="pkf", bufs=2) as pkf, \
                   tc.tile_pool(name="psmall", bufs=1) as psmall, \
                   tc.tile_pool(name="pksq", bufs=1) as pksq, \
                   tc.tile_pool(name="psumA", bufs=4, space="PSUM") as pA, \
                   tc.tile_pool(name="psumB", bufs=2, space="PSUM") as pB:

                  def proj_block(src, blk_i, is_q):
                      # load + transpose x block [512, D] -> xT [dim, tok] bf16
                      xst = pstage.tile([128, 4, D], F32, tag="xst")
                      nc.sync.dma_start(
                          out=xst,
                          in_=src[blk_i * BLK:(blk_i + 1) * BLK, :].rearrange(
                              "(t p) d -> p t d", p=128),
                      )
                      xT = pxT.tile([128, 6, BLK], BF16, tag="xT")
                      for dt in range(6):
                          tp = pA.tile([128, 512], F32, tag="pA")
                          for tt in range(4):
                              nc.tensor.transpose(
                                  tp[:, tt * 128:(tt + 1) * 128],
                                  xst[:, tt, dt * 128:(dt + 1) * 128], ident)
                          nc.vector.tensor_copy(xT[:, dt, :], tp)

                      wbase = 0 if is_q else INNER
                      # q^T / k^T projection [dim_out, tok]
                      kf = pkf.tile([128, 6, BLK], BF16, tag="kf")
                      for dt in range(6):
                          kp = pA.tile([128, 512], F32, tag="pA")
                          for ks in range(6):
                              nc.tensor.matmul(
                                  kp,
                                  wq[:, ks, wbase + dt * 128: wbase + (dt + 1) * 128],
                                  xT[:, ks, :],
                                  start=(ks == 0), stop=(ks == 5))
                          nc.vector.tensor_copy(kf[:, dt, :], kp)
                      # ssq over heads: sel matmul on squares
                      sq = pA.tile([128, 512], F32, tag="pA")
                      for dt in range(6):
                          ksq = pksq.tile([128, BLK], BF16, tag="ksq")
                          nc.vector.tensor_mul(ksq, kf[:, dt, :], kf[:, dt, :])
                          nc.tensor.matmul(sq, sel_bf, ksq,
                                           start=(dt == 0), stop=(dt == 5))
                      nrm = psmall.tile([128, BLK], F32, tag="nrm")
                      nc.scalar.activation(nrm, sq, mybir.ActivationFunctionType.Sqrt)
                      u = psmall.tile([128, BLK], F32, tag="u")
                      nc.scalar.activation(u, nrm, mybir.ActivationFunctionType.Sqrt,
                                           bias=eps_t[:, :])
                      rq = psmall.tile([128, BLK], F32, tag="rq")
                      nc.vector.reciprocal(rq, u)
                      bsl = bass.ts(blk_i, BLK)
                      if is_q:
                          # zero-padded per-head layout: head 2dt on rows 0:64,
                          # head 2dt+1 on rows 64:128, other rows stay zero.
                          # 1/scale_h is folded into qhat so exp needs no scale.
                          for dt in range(6):
                              a = qhat[0:64, 2 * dt, bsl]
                              b = qhat[64:128, 2 * dt + 1, bsl]
                              nc.vector.tensor_mul(a, kf[0:64, dt, :], rq[0:64, :])
                              nc.vector.tensor_mul(b, kf[64:128, dt, :],
                                                   rq[64:128, :])
                              nc.vector.tensor_scalar_mul(a, a,
                                                          invs[0:64, dt:dt + 1])
                              nc.vector.tensor_scalar_mul(b, b,
                                                          invs[64:128, dt:dt + 1])
                      else:
                          for dt in range(6):
                              nc.vector.tensor_mul(
                                  khat[:, dt, bsl], kf[:, dt, :], rq)
                      if is_q:
                          return
                      # v projection [tok, inner] -> vhat strided 65
                      for tt in range(4):
                          vp = pB.tile([128, 1024], F32, tag="pB")
                          for ks in range(6):
                              nc.tensor.matmul(vp[:, 0:512],
                                               xT[:, ks, tt * 128:(tt + 1) * 128],
                                               wq[:, ks, 2 * INNER:2 * INNER + 512],
                                               start=(ks == 0), stop=(ks == 5))
                              nc.tensor.matmul(vp[:, 512:768],
                                               xT[:, ks, tt * 128:(tt + 1) * 128],
                                               wq[:, ks, 2 * INNER + 512:3 * INNER],
                                               start=(ks == 0), stop=(ks == 5))
                          vdst = vhat[:, blk_i * 4 + tt, :].rearrange(
                              "p (h c) -> p h c", c=65)[:, :, 0:64]
                          nc.vector.tensor_copy(
                              vdst, vp[:, 0:768].rearrange("p (h c) -> p h c", c=64))

                  for blk in range(NQ // BLK):
                      proj_block(qx, blk, True)
                  for blk in range(N // BLK):
                      proj_block(xb, blk, False)

            # ---------------- phase A: attention ----------------
            with tc.tile_pool(name="opersist", bufs=1) as operp:
              oh_all = operp.tile([64, H, NQ], BF16)
              wo12 = operp.tile([64, H, D], BF16)
              with tc.tile_pool(name="wostage", bufs=2) as wost:
                for h in range(H):
                    wst_t = wost.tile([64, D], F32, tag="wost")
                    nc.sync.dma_start(out=wst_t, in_=woT[h * 64:(h + 1) * 64, :])
                    nc.vector.tensor_copy(wo12[:, h, :], wst_t)
              with tc.tile_pool(name="pP", bufs=6) as pP, \
                   tc.tile_pool(name="poraw", bufs=6) as poraw, \
                   tc.tile_pool(name="princ", bufs=2) as princ, \
                   tc.tile_pool(name="psumS", bufs=2, space="PSUM") as pS, \
                   tc.tile_pool(name="psumO", bufs=4, space="PSUM") as pO:
                  # head-pair processing: heads (2i, 2i+1) live on PE row
                  # groups 0-63 / 64-127 and run concurrently. Queries are
                  # split in 512-halves so every PSUM tile is one bank.
                  for hp in range(6):
                      hs = (2 * hp, 2 * hp + 1)
                      ots = {}
                      for h in hs:
                          for qh in range(2):
                              ot = pO.tile([65, 512], F32, tag="pO",
                                           name=f"ot_{h}_{qh}")
                              ots[(h, qh)] = ot
                      for kb in range(KT):
                          for qh in range(2):
                              qsl = bass.ts(qh, 512)
                              st = pS.tile([128, 1024], F32, tag="pS",
                                           name=f"st_{qh}")
                              for j, h in enumerate(hs):
                                  nc.tensor.matmul(
                                      st[:, j * 512:(j + 1) * 512],
                                      khat[:, hp, kb * 128:(kb + 1) * 128],
                                      qhat[:, h, qsl],
                                      start=True, stop=True)
                              pt = pP.tile([128, 1024], BF16, tag="pP",
                                           name=f"pt_{qh}")
                              nc.scalar.activation(
                                  pt, st, mybir.ActivationFunctionType.Exp)
                              for j, h in enumerate(hs):
                                  nc.tensor.matmul(
                                      ots[(h, qh)],
                                      vhat[:, kb, h * 65:(h + 1) * 65],
                                      pt[:, j * 512:(j + 1) * 512],
                                      start=(kb == 0), stop=(kb == KT - 1))
                      for h in hs:
                          for qh in range(2):
                              qsl = bass.ts(qh, 512)
                              o_raw = poraw.tile([65, 512], F32, tag="oraw",
                                                 name=f"oraw_{h}_{qh}")
                              if h % 2 == 0:
                                  nc.vector.tensor_copy(o_raw, ots[(h, qh)])
                              else:
                                  nc.scalar.copy(o_raw, ots[(h, qh)])
                              rinv = princ.tile([1, 512], F32, tag="rinv",
                                                name=f"rinv_{h}_{qh}")
                              nc.vector.reciprocal(rinv, o_raw[64:65, :])
                              rbc = pS.tile([128, 512], F32, tag="pS",
                                            name=f"rbc_{h}_{qh}")
                              nc.tensor.matmul(rbc[0:64, :], ones_f, rinv,
                                               start=True, stop=True)
                              nc.vector.tensor_mul(oh_all[:, h, qsl],
                                                   o_raw[0:64, :], rbc[0:64, :])

            # ---------------- phase Y: output projection ----------------
              with tc.tile_pool(name="pys", bufs=2) as pys, \
                   tc.tile_pool(name="psumY", bufs=2, space="PSUM") as pY:
                  for mt in range(NQ // 128):
                      yp = pY.tile([128, 1024], F32, tag="pY")
                      for h in range(H):
                          lhsT = oh_all[:, h, mt * 128:(mt + 1) * 128]
                          nc.tensor.matmul(yp[:, 0:512], lhsT, wo12[:, h, 0:512],
                                           start=(h == 0), stop=False)
                          nc.tensor.matmul(yp[:, 512:768], lhsT, wo12[:, h, 512:768],
                                           start=(h == 0), stop=False)
                      nc.tensor.matmul(yp[:, 0:512], ones_bf, b_bf[:, 0:512],
                                       start=False, stop=True)
                      nc.tensor.matmul(yp[:, 512:768], ones_bf, b_bf[:, 512:768],
                                       start=False, stop=True)
                      ys = pys.tile([128, D], F32, tag="ys")
                      nc.vector.tensor_copy(ys, yp[:, 0:768])
                      nc.sync.dma_start(out=y[mt * 128:(mt + 1) * 128, :], in_=ys)

    _split_multi_waits(nc)
    return nc


_prog_cache = {}


def kernel(x, w_qkv, w_out, b_out, scale):
    x = np.ascontiguousarray(np.asarray(x, dtype=np.float32))
    w_qkv = np.asarray(w_qkv, dtype=np.float32)
    w_out = np.asarray(w_out, dtype=np.float32)
    b_out = np.asarray(b_out, dtype=np.float32).reshape(1, D)
    scale = np.asarray(scale, dtype=np.float32)

    inv_scale = tuple(float(1.0 / s) for s in scale)
    nc = _prog_cache.get(inv_scale)
    if nc is None:
        nc = _build_program(inv_scale)
        _prog_cache[inv_scale] = nc

    wqkvT = np.ascontiguousarray(w_qkv.T)            # [768, 2304]
    woT = np.ascontiguousarray(w_out.T)              # [768, 768]
    p = np.arange(128)
    sel = (p[:, None] % 64 == p[None, :] % 64).astype(np.float32)

    in_maps = []
    for c in range(NCORES):
        bi, qi = c // 4, c % 4
        in_maps.append({
            "xb": x[bi],
            "qx": np.ascontiguousarray(x[bi, qi * NQ:(qi + 1) * NQ]),
            "wqkvT": wqkvT,
            "woT": woT,
            "bout": b_out,
            "selin": sel,
        })

    res = run_bass_kernel_spmd(nc, in_maps, core_ids=list(range(NCORES)))
    out = np.empty((B, N, D), dtype=np.float32)
    for c in range(NCORES):
        bi, qi = c // 4, c % 4
        out[bi, qi * NQ:(qi + 1) * NQ] = res.results[c]["y"]
    return out



# revision 2
# speedup vs baseline: 1.0084x; 1.0084x over previous
"""CosineSimilarityAttention Trainium2 kernel (8 NeuronCores, SPMD).

Sharding: token-parallel. Global tokens = 2 batches x 4096. Core c handles
batch (c // 4), query rows (c % 4)*1024 .. +1024. Each core computes K/V
projections for its whole batch (4096 tokens) -- replicated within each
4-core batch group -- plus Q for its own 1024 tokens, then attention and
the output projection for its token slice. Outputs concatenate on host.

Math per batch (faithful to reference):
  qkv = x @ w_qkv.T ; split q,k,v ; reshape heads h=12, dh=64
  q *= 1/sqrt(||q||_heads + eps)   (L2 norm over the HEADS axis, per (n, dh))
  k *= 1/sqrt(||k||_heads + eps)
  out_h = softmax((q_h k_h^T) / scale_h) v_h   (no max-subtract: |logits|<~2)
  y = concat_h(out_h) @ w_out.T + b_out
"""

import numpy as np

import concourse.bass as bass
import concourse.mybir as mybir
import concourse.tile as tile
from concourse.bass_utils import run_bass_kernel_spmd
from concourse.masks import make_identity

F32 = mybir.dt.float32
BF16 = mybir.dt.bfloat16

B = 2
N = 4096          # tokens per batch
D = 768           # model dim
H = 12            # heads
DH = 64           # head dim
INNER = H * DH    # 768
EPS = 1e-8
NQ = 1024         # query tokens per core
NCORES = 8
BLK = 512         # projection token block
KT = N // 128     # 32 key tiles of 128


def _split_multi_waits(nc):
    """This container's walrus accepts only ONE sync-wait per instruction.
    Hoist extra waits into standalone EVSEM instructions placed just before."""
    n = 0
    for f in nc.m.functions:
        for bb in f.blocks:
            insts = list(bb.instructions)
            out = []
            for inst in insts:
                si = inst.sync_info
                if si is not None and si.on_wait is not None and len(si.on_wait) > 1:
                    waits = list(si.on_wait)
                    for j, w in enumerate(waits[:-1]):
                        ev = mybir.InstEventSemaphore(
                            name=f"{inst.name}-evw{j}",
                            engine=inst.engine,
                            sync_info=mybir.SyncInfo(on_wait=[w], on_update=[]),
                        )
                        out.append(ev)
                        n += 1
                    si.on_wait = [waits[-1]]
                out.append(inst)
            bb.instructions = out
    return n


def _build_program(inv_scale):
    """Build the single SPMD Bass program. inv_scale: list of 12 floats."""
    nc = bass.Bass()
    xb = nc.declare_dram_parameter("xb", [N, D], F32, isOutput=False)
    qx = nc.declare_dram_parameter("qx", [NQ, D], F32, isOutput=False)
    wqkvT = nc.declare_dram_parameter("wqkvT", [D, 3 * INNER], F32, isOutput=False)
    woT = nc.declare_dram_parameter("woT", [INNER, D], F32, isOutput=False)
    bout = nc.declare_dram_parameter("bout", [1, D], F32, isOutput=False)
    selin = nc.declare_dram_parameter("selin", [128, 128], F32, isOutput=False)
    y = nc.declare_dram_parameter("y", [NQ, D], F32, isOutput=True)

    with tile.TileContext(nc) as tc:
        with tc.tile_pool(name="const", bufs=1) as constp, \
             tc.tile_pool(name="persist", bufs=1) as persist:
            # --- constants ---
            ident = constp.tile([128, 128], F32)
            make_identity(nc, ident)
            sel_st = constp.tile([128, 128], F32)
            nc.sync.dma_start(out=sel_st, in_=selin[:, :])
            sel_bf = constp.tile([128, 128], BF16)
            nc.vector.tensor_copy(sel_bf, sel_st)
            ones_f = constp.tile([1, 64], F32)
            nc.vector.memset(ones_f, 1.0)
            ones_bf = constp.tile([1, 128], BF16)
            nc.vector.memset(ones_bf, 1.0)
            eps_t = constp.tile([128, 1], F32)
            nc.vector.memset(eps_t, EPS)
            invs = constp.tile([128, 6], F32)
            for dt in range(6):
                nc.vector.memset(invs[0:64, dt:dt + 1], float(inv_scale[2 * dt]))
                nc.vector.memset(invs[64:128, dt:dt + 1], float(inv_scale[2 * dt + 1]))
            b_st = constp.tile([1, D], F32)
            nc.sync.dma_start(out=b_st, in_=bout[:, :])
            b_bf = constp.tile([1, D], BF16)
            nc.vector.tensor_copy(b_bf, b_st)

            # --- persistent activations ---
            khat = persist.tile([128, 6, N], BF16)     # k^T normalized  [dim, tok]
            qhat = persist.tile([128, H, NQ], BF16)    # q^T per head, K=128 zero-padded
            vhat = persist.tile([128, KT, H * 65], BF16)  # v [tok, h*65] (+ones col)

            # ones columns of vhat (col 64 of every 65-block)
            vones = vhat.rearrange("p t (h c) -> p t h c", c=65)[:, :, :, 64:65]
            nc.vector.memset(vones, 1.0)
            nc.vector.memset(qhat, 0.0)

            # ---------------- phase W+P: weights, projections, head-norm ----
            with tc.tile_pool(name="pw", bufs=1) as pwp:
              wq = pwp.tile([128, 6, 3 * INNER], BF16)
              with tc.tile_pool(name="wstage", bufs=1) as wst:
                for dt in range(6):
                    st = wst.tile([128, 3 * INNER], F32, tag="wst")
                    nc.sync.dma_start(out=st, in_=wqkvT[dt * 128:(dt + 1) * 128, :])
                    nc.vector.tensor_copy(wq[:, dt, :], st)
              with tc.tile_pool(name="pstage", bufs=1) as pstage, \
                   tc.tile_pool(name="pxT", bufs=2) as pxT, \
                   tc.tile_pool(name="pkf", bufs=2) as pkf, \
                   tc.tile_pool(name="psmall", bufs=1) as psmall, \
                   tc.tile_pool(name="pksq", bufs=1) as pksq, \
                   tc.tile_pool(name="psumA", bufs=4, space="PSUM") as pA, \
                   tc.tile_pool(name="psumB", bufs=2, space="PSUM") as pB:

                  def proj_block(src, blk_i, is_q):
                      # load + transpose x block [512, D] -> xT [dim, tok] bf16
                      xst = pstage.tile([128, 4, D], F32, tag="xst")
                      nc.sync.dma_start(
                          out=xst,
                          in_=src[blk_i * BLK:(blk_i + 1) * BLK, :].rearrange(
                              "(t p) d -> p t d", p=128),
                      )
                      xT = pxT.tile([128, 6, BLK], BF16, tag="xT")
                      for dt in range(6):
                          tp = pA.tile([128, 512], F32, tag="pA")
                          for tt in range(4):
                              nc.tensor.transpose(
                                  tp[:, tt * 128:(tt + 1) * 128],
                                  xst[:, tt, dt * 128:(dt + 1) * 128], ident)
                          nc.vector.tensor_copy(xT[:, dt, :], tp)

                      wbase = 0 if is_q else INNER
                      # q^T / k^T projection [dim_out, tok]
                      kf = pkf.tile([128, 6, BLK], BF16, tag="kf")
                      for dt in range(6):
                          kp = pA.tile([128, 512], F32, tag="pA")
                          for ks in range(6):
                              nc.tensor.matmul(
                                  kp,
                                  wq[:, ks, wbase + dt * 128: wbase + (dt + 1) * 128],
                                  xT[:, ks, :],
                                  start=(ks == 0), stop=(ks == 5))
                          nc.vector.tensor_copy(kf[:, dt, :], kp)
                      # ssq over heads: sel matmul on squares
                      sq = pA.tile([128, 512], F32, tag="pA")
                      for dt in range(6):
                          ksq = pksq.tile([128, BLK], BF16, tag="ksq")
                          nc.vector.tensor_mul(ksq, kf[:, dt, :], kf[:, dt, :])
                          nc.tensor.matmul(sq, sel_bf, ksq,
                                           start=(dt == 0), stop=(dt == 5))
                      nrm = psmall.tile([128, BLK], F32, tag="nrm")
                      nc.scalar.activation(nrm, sq, mybir.ActivationFunctionType.Sqrt)
                      u = psmall.tile([128, BLK], F32, tag="u")
                      nc.scalar.activation(u, nrm, mybir.ActivationFunctionType.Sqrt,
                                           bias=eps_t[:, :])
                      rq = psmall.tile([128, BLK], F32, tag="rq")
                      nc.vector.reciprocal(rq, u)
                      bsl = bass.ts(blk_i, BLK)
                      if is_q:
                          # zero-padded per-head layout: head 2dt on rows 0:64,
                          # head 2dt+1 on rows 64:128, other rows stay zero.
                          # 1/scale_h is folded into qhat so exp needs no scale.
                          for dt in range(6):
                              a = qhat[0:64, 2 * dt, bsl]
                              b = qhat[64:128, 2 * dt + 1, bsl]
                              nc.vector.tensor_mul(a, kf[0:64, dt, :], rq[0:64, :])
                              nc.vector.tensor_mul(b, kf[64:128, dt, :],
                                                   rq[64:128, :])
                              nc.vector.tensor_scalar_mul(a, a,
                                                          invs[0:64, dt:dt + 1])
                              nc.vector.tensor_scalar_mul(b, b,
                                                          invs[64:128, dt:dt + 1])
                      else:
                          for dt in range(6):
                              nc.vector.tensor_mul(
                                  khat[:, dt, bsl], kf[:, dt, :], rq)
                      if is_q:
                          return
                      # v projection [tok, inner] -> vhat strided 65
                      for tt in range(4):
                          vp = pB.tile([128, 1024], F32, tag="pB")
                          for ks in range(6):
                              nc.tensor.matmul(vp[:, 0:512],
                                               xT[:, ks, tt * 128:(tt + 1) * 128],
                                               wq[:, ks, 2 * INNER:2 * INNER + 512],
                                               start=(ks == 0), stop=(ks == 5))
                              nc.tensor.matmul(vp[:, 512:768],
                                               xT[:, ks, tt * 128:(tt + 1) * 128],
                                               wq[:, ks, 2 * INNER + 512:3 * INNER],
                                               start=(ks == 0), stop=(ks == 5))
                          vdst = vhat[:, blk_i * 4 + tt, :].rearrange(
                              "p (h c) -> p h c", c=65)[:, :, 0:64]
                          nc.vector.tensor_copy(
                              vdst, vp[:, 0:768].rearrange("p (h c) -> p h c", c=64))

                  for blk in range(NQ // BLK):
                      proj_block(qx, blk, True)
                  for blk in range(N // BLK):
                      proj_block(xb, blk, False)

            # ---------------- phase A: attention ----------------
            with tc.tile_pool(name="opersist", bufs=1) as operp:
              oh_all = operp.tile([64, H, NQ], BF16)
              wo12 = operp.tile([64, H, D], BF16)
              with tc.tile_pool(name="wostage", bufs=2) as wost:
                for h in range(H):
                    wst_t = wost.tile([64, D], F32, tag="wost")
                    nc.sync.dma_start(out=wst_t, in_=woT[h * 64:(h + 1) * 64, :])
                    nc.vector.tensor_copy(wo12[:, h, :], wst_t)
              with tc.tile_pool(name="pP", bufs=6) as pP, \
                   tc.tile_pool(name="poraw", bufs=6) as poraw, \
                   tc.tile_pool(name="princ", bufs=2) as princ, \
                   tc.tile_pool(name="psumS", bufs=2, space="PSUM") as pS, \
                   tc.tile_pool(name="psumO", bufs=4, space="PSUM") as pO:
                  # head-pair processing: heads (2i, 2i+1) live on PE row
                  # groups 0-63 / 64-127 and run concurrently. Queries are
                  # split in 512-halves so every PSUM tile is one bank.
                  for hp in range(6):
                      hs = (2 * hp, 2 * hp + 1)
                      ots = {}
                      for h in hs:
                          for qh in range(2):
                              ot = pO.tile([65, 512], F32, tag="pO",
                                           name=f"ot_{h}_{qh}")
                              ots[(h, qh)] = ot
                      for kb in range(KT):
                          for qh in range(2):
                              qsl = bass.ts(qh, 512)
                              st = pS.tile([128, 1024], F32, tag="pS",
                                           name=f"st_{qh}")
                              for j, h in enumerate(hs):
                                  nc.tensor.matmul(
                                      st[:, j * 512:(j + 1) * 512],
                                      khat[:, hp, kb * 128:(kb + 1) * 128],
                                      qhat[:, h, qsl],
                                      start=True, stop=True)
                              pt = pP.tile([128, 1024], BF16, tag="pP",
                                           name=f"pt_{qh}")
                              nc.scalar.activation(
                                  pt, st, mybir.ActivationFunctionType.Exp)
                              for j, h in enumerate(hs):
                                  nc.tensor.matmul(
                                      ots[(h, qh)],
                                      vhat[:, kb, h * 65:(h + 1) * 65],
                                      pt[:, j * 512:(j + 1) * 512],
                                      start=(kb == 0), stop=(kb == KT - 1))
                      for h in hs:
                          for qh in range(2):
                              qsl = bass.ts(qh, 512)
                              o_raw = poraw.tile([65, 512], F32, tag="oraw",
                                                 name=f"oraw_{h}_{qh}")
                              if h % 2 == 0:
                                  nc.vector.tensor_copy(o_raw, ots[(h, qh)])
                              else:
                                  nc.scalar.copy(o_raw, ots[(h, qh)])
                              rinv = princ.tile([1, 512], F32, tag="rinv",
                                                name=f"rinv_{h}_{qh}")
                              nc.vector.reciprocal(rinv, o_raw[64:65, :])
                              rbc = pS.tile([128, 512], F32, tag="pS",
                                            name=f"rbc_{h}_{qh}")
                              nc.tensor.matmul(rbc[0:64, :], ones_f, rinv,
                                               start=True, stop=True)
                              nc.vector.tensor_mul(oh_all[:, h, qsl],
                                                   o_raw[0:64, :], rbc[0:64, :])

            # ---------------- phase Y: output projection ----------------
              with tc.tile_pool(name="pys", bufs=2) as pys, \
                   tc.tile_pool(name="psumY", bufs=2, space="PSUM") as pY:
                  for mt in range(NQ // 128):
                      yp = pY.tile([128, 1024], F32, tag="pY")
                      for h in range(H):
                          lhsT = oh_all[:, h, mt * 128:(mt + 1) * 128]
                          nc.tensor.matmul(yp[:, 0:512], lhsT, wo12[:, h, 0:512],
                                           start=(h == 0), stop=False)
                          nc.tensor.matmul(yp[:, 512:768], lhsT, wo12[:, h, 512:768],
                                           start=(h == 0), stop=False)
                      nc.tensor.matmul(yp[:, 0:512], ones_bf, b_bf[:, 0:512],
                                       start=False, stop=True)
                      nc.tensor.matmul(yp[:, 512:768], ones_bf, b_bf[:, 512:768],
                                       start=False, stop=True)
                      ys = pys.tile([128, D], F32, tag="ys")
                      nc.vector.tensor_copy(ys, yp[:, 0:768])
                      nc.sync.dma_start(out=y[mt * 128:(mt + 1) * 128, :], in_=ys)

    _split_multi_waits(nc)
    return nc


_prog_cache = {}


def _make_in_maps(inputs):
    x = np.ascontiguousarray(np.asarray(inputs["x"], dtype=np.float32))
    w_qkv = np.asarray(inputs["w_qkv"], dtype=np.float32)
    w_out = np.asarray(inputs["w_out"], dtype=np.float32)
    b_out = np.asarray(inputs["b_out"], dtype=np.float32).reshape(1, D)
    wqkvT = np.ascontiguousarray(w_qkv.T)            # [768, 2304]
    woT = np.ascontiguousarray(w_out.T)              # [768, 768]
    p = np.arange(128)
    sel = (p[:, None] % 64 == p[None, :] % 64).astype(np.float32)

    in_maps = []
    for c in range(NCORES):
        bi, qi = c // 4, c % 4
        in_maps.append({
            "xb": x[bi],
            "qx": np.ascontiguousarray(x[bi, qi * NQ:(qi + 1) * NQ]),
            "wqkvT": wqkvT,
            "woT": woT,
            "bout": b_out,
            "selin": sel,
        })
    return in_maps


def kernel(x, w_qkv, w_out, b_out, scale):
    scale = np.asarray(scale, dtype=np.float32)
    inv_scale = tuple(float(1.0 / s) for s in scale)
    nc = _prog_cache.get(inv_scale)
    if nc is None:
        nc = _build_program(inv_scale)
        _prog_cache[inv_scale] = nc

    in_maps = _make_in_maps(
        {"x": x, "w_qkv": w_qkv, "w_out": w_out, "b_out": b_out})

    res = run_bass_kernel_spmd(nc, in_maps, core_ids=list(range(NCORES)))
    out = np.empty((B, N, D), dtype=np.float32)
    for c in range(NCORES):
        bi, qi = c // 4, c % 4
        out[bi, qi * NQ:(qi + 1) * NQ] = res.results[c]["y"]
    return out



# revision 16
# speedup vs baseline: 1.3383x; 1.3271x over previous
"""CosineSimilarityAttention Trainium2 kernel (8 NeuronCores, SPMD).

v2: chunk-pipelined design. Global tokens = 2 batches x 4096; core c handles
batch (c // 4) and query rows (c % 4)*1024 .. +1024. The host rotates each
core's batch so its query tokens are rows 0:1024, and pre-converts x and the
weights to bf16 so the kernel DMA-transposes x straight from DRAM.

Per chunk of 1024 key tokens (4 chunks):
  P(c): xT via DMA-transpose; k (and, on chunk 0, q) projections + the
        head-axis norm  k * (ssq)^-1/4  computed as exp(-0.25*ln(ssq)) so the
        scalar engine stays on one activation table set; v projection into a
        65-stride layout with a ones column (softmax denominator trick).
  A(c): for each head pair hp and query half qh: 8x [K=64 row-tiled score
        matmul pair] -> exp -> AV accumulate in PSUM; then add into an SBUF
        accumulator.  P(c+1) is emitted before A(c) so projections fill
        engine gaps under the exp-bound attention stream.
Tail: softmax denominators -> fast reciprocal -> ones-matmul broadcast ->
      normalize -> output projection (row-tiled pairs) + bias -> DMA out.
"""

import numpy as np
import ml_dtypes

import concourse.bass as bass
import concourse.mybir as mybir
import concourse.tile as tile
from concourse.bass_utils import run_bass_kernel_spmd

F32 = mybir.dt.float32
BF16 = mybir.dt.bfloat16
AF = mybir.ActivationFunctionType

B = 2
N = 4096          # tokens per batch
D = 768           # model dim
H = 12            # heads
DH = 64           # head dim
INNER = H * DH    # 768
NQ = 1024         # query tokens per core
NCORES = 8
CH = 1024         # key-chunk tokens
NCH = N // CH     # 4 chunks
KBC = CH // 128   # 8 key tiles of 128 per chunk
BLK = 512         # projection token block (2 per chunk)


def _split_multi_waits(nc):
    """This container's walrus accepts only ONE sync-wait per instruction.
    Hoist extra waits into standalone EVSEM instructions placed just before."""
    n = 0
    for f in nc.m.functions:
        for bb in f.blocks:
            insts = list(bb.instructions)
            out = []
            for inst in insts:
                si = inst.sync_info
                if si is not None and si.on_wait is not None and len(si.on_wait) > 1:
                    waits = list(si.on_wait)
                    for j, w in enumerate(waits[:-1]):
                        ev = mybir.InstEventSemaphore(
                            name=f"{inst.name}-evw{j}",
                            engine=inst.engine,
                            sync_info=mybir.SyncInfo(on_wait=[w], on_update=[]),
                        )
                        out.append(ev)
                        n += 1
                    si.on_wait = [waits[-1]]
                out.append(inst)
            bb.instructions = out
    return n


def _build_program(inv_scale, debug=False):
    nc = bass.Bass()
    xb = nc.declare_dram_parameter("xb", [N, D], BF16, isOutput=False)
    if debug:
        dbg_qhat = nc.declare_dram_parameter("dbg_qhat", [128, 6 * NQ], BF16,
                                             isOutput=True)
        dbg_khat = nc.declare_dram_parameter("dbg_khat", [128, 6 * CH], BF16,
                                             isOutput=True)
        dbg_vhat = nc.declare_dram_parameter("dbg_vhat", [128, KBC * H * 65],
                                             BF16, isOutput=True)
        dbg_acc = nc.declare_dram_parameter("dbg_acc", [65, H * 2 * 512], BF16,
                                            isOutput=True)
        dbg_ohp = nc.declare_dram_parameter("dbg_ohp", [128, 6 * NQ], BF16,
                                            isOutput=True)
    wqkv = nc.declare_dram_parameter("wqkv", [128, 6 * 3 * INNER], BF16,
                                     isOutput=False)
    wo_in = nc.declare_dram_parameter("wo", [128, 6 * D], BF16, isOutput=False)
    bout = nc.declare_dram_parameter("bout", [1, D], BF16, isOutput=False)
    selin = nc.declare_dram_parameter("selin", [128, 128], BF16, isOutput=False)
    y = nc.declare_dram_parameter("y", [NQ, D], F32, isOutput=True)

    with tile.TileContext(nc) as tc:
        with tc.tile_pool(name="const", bufs=1) as constp, \
             tc.tile_pool(name="persist", bufs=1) as persist, \
             tc.tile_pool(name="pkh", bufs=2) as pkh, \
             tc.tile_pool(name="pvh", bufs=2) as pvh, \
             tc.tile_pool(name="pxT", bufs=1) as pxT, \
             tc.tile_pool(name="pkf", bufs=2) as pkf, \
             tc.tile_pool(name="pksq", bufs=1) as pksq, \
             tc.tile_pool(name="pnrm", bufs=2) as pnrm, \
             tc.tile_pool(name="ppt", bufs=4) as ppt, \
             tc.tile_pool(name="pys", bufs=2) as pys, \
             tc.tile_pool(name="psS", bufs=2, space="PSUM") as pS, \
             tc.tile_pool(name="psO", bufs=2, space="PSUM") as pO, \
             tc.tile_pool(name="psP", bufs=2, space="PSUM") as pP:

            # ---- constants / weights ----
            sel_bf = constp.tile([128, 128], BF16)
            nc.sync.dma_start(out=sel_bf, in_=selin[:, :])
            b_bf = constp.tile([1, D], BF16)
            nc.sync.dma_start(out=b_bf, in_=bout[:, :])
            ones33 = constp.tile([33, 64], F32)
            nc.vector.memset(ones33, 1.0)
            ones_bf = constp.tile([1, 128], BF16)
            nc.vector.memset(ones_bf, 1.0)
            invs = constp.tile([128, 6], F32)
            for dt in range(6):
                nc.vector.memset(invs[0:64, dt:dt + 1], float(inv_scale[2 * dt]))
                nc.vector.memset(invs[64:128, dt:dt + 1],
                                 float(inv_scale[2 * dt + 1]))
            wq = constp.tile([128, 6, 3 * INNER], BF16)
            nc.sync.dma_start(out=wq, in_=wqkv[:, :].rearrange(
                "p (k c) -> p k c", k=6))
            wo = constp.tile([128, 6, D], BF16)
            nc.sync.dma_start(out=wo, in_=wo_in[:, :].rearrange(
                "p (k c) -> p k c", k=6))

            # ---- persistent activations ----
            qhat = persist.tile([128, 6, NQ], BF16)     # pair layout, 1/scale folded
            acc = persist.tile([65, H, 2, 512], BF16)   # rows 0:64 num, row 64 den
            ohp = persist.tile([128, 6, NQ], BF16)      # normalized out, pair layout

            # ================= projection of one chunk =================
            def proj_chunk(c, with_q):
                xT = pxT.tile([128, 6, CH], BF16, tag="xT")
                for dt in range(6):
                    nc.sync.dma_start(
                        out=xT[:, dt, :],
                        in_=xb[c * CH:(c + 1) * CH, dt * 128:(dt + 1) * 128],
                        transpose=True)
                khat = pkh.tile([128, 6, CH], BF16, tag="khat")
                vhat = pvh.tile([128, KBC, H * 65], BF16, tag="vhat")
                vones = vhat.rearrange("p t (h c) -> p t h c", c=65)[:, :, :, 64:65]
                nc.vector.memset(vones, 1.0)

                for blk in range(CH // BLK):
                    bsl = bass.ts(blk, BLK)

                    def qk_side(wbase, is_q):
                        kf = pkf.tile([128, 6, BLK], BF16, tag="kf")
                        ksq = pksq.tile([128, 6, BLK], BF16, tag="ksq")
                        for dt in range(6):
                            kp = pP.tile([128, BLK], F32, tag="pP")
                            for ks in range(6):
                                nc.tensor.matmul(
                                    kp,
                                    wq[:, ks, wbase + dt * 128:
                                       wbase + (dt + 1) * 128],
                                    xT[:, ks, bsl],
                                    start=(ks == 0), stop=(ks == 5))
                            nc.vector.tensor_copy(kf[:, dt, :], kp)
                            nc.vector.tensor_mul(ksq[:, dt, :], kf[:, dt, :],
                                                 kf[:, dt, :])
                        sq = pP.tile([128, BLK], F32, tag="pP")
                        for dt in range(6):
                            nc.tensor.matmul(sq, sel_bf, ksq[:, dt, :],
                                             start=(dt == 0), stop=(dt == 5))
                        # rq = ssq^(-1/4) = exp(-0.25 * ln(ssq)); eps dropped
                        # (||k|| ~ 3.5 >> 1e-8).  Ln+Exp share one ACT table set.
                        lnt = pnrm.tile([128, BLK], F32, tag="lnt")
                        nc.scalar.activation(lnt, sq, AF.Ln)
                        rq = pnrm.tile([128, BLK], F32, tag="rq")
                        nc.scalar.activation(rq, lnt, AF.Exp, scale=-0.25)
                        for dt in range(6):
                            dst = qhat[:, dt, bsl] if is_q else khat[:, dt, bsl]
                            nc.vector.tensor_mul(dst, kf[:, dt, :], rq)
                            if is_q:
                                nc.vector.tensor_scalar_mul(
                                    dst, dst, invs[:, dt:dt + 1])

                    qk_side(INNER, False)
                    if with_q:
                        qk_side(0, True)

                    for tt in range(4):
                        vp1 = pP.tile([128, BLK], F32, tag="pP")
                        vp2 = pP.tile([128, 256], F32, tag="pP")
                        tsl = bass.ds(blk * BLK + tt * 128, 128)
                        for ks in range(6):
                            nc.tensor.matmul(vp1, xT[:, ks, tsl],
                                             wq[:, ks, 2 * INNER:2 * INNER + 512],
                                             start=(ks == 0), stop=(ks == 5))
                        for ks in range(6):
                            nc.tensor.matmul(vp2, xT[:, ks, tsl],
                                             wq[:, ks, 2 * INNER + 512:3 * INNER],
                                             start=(ks == 0), stop=(ks == 5))
                        vdst = vhat[:, blk * 4 + tt, :].rearrange(
                            "p (h c) -> p h c", c=65)
                        nc.vector.tensor_copy(
                            vdst[:, 0:8, 0:64],
                            vp1.rearrange("p (h c) -> p h c", c=64))
                        nc.vector.tensor_copy(
                            vdst[:, 8:12, 0:64],
                            vp2.rearrange("p (h c) -> p h c", c=64))
                return khat, vhat

            # ================= attention over one chunk =================
            def attn_chunk(ci, khat, vhat, first, last):
                for hp in range(6):
                    ha, hb = 2 * hp, 2 * hp + 1
                    for qh in range(2):
                        qsl = bass.ts(qh, 512)
                        ota = pO.tile([65, 512], F32, tag="ot",
                                      name=f"ota_{ci}_{hp}_{qh}")
                        otb = pO.tile([65, 512], F32, tag="ot",
                                      name=f"otb_{ci}_{hp}_{qh}")
                        for kb in range(KBC):
                            ksl = bass.ts(kb, 128)
                            st = pS.tile([128, 1024], F32, tag="pS")
                            nc.tensor.matmul(st[:, 0:512],
                                             khat[0:64, hp, ksl],
                                             qhat[0:64, hp, qsl],
                                             start=True, stop=True,
                                             tile_position=(0, 0))
                            nc.tensor.matmul(st[:, 512:1024],
                                             khat[64:128, hp, ksl],
                                             qhat[64:128, hp, qsl],
                                             start=True, stop=True,
                                             tile_position=(64, 0))
                            pt = ppt.tile([128, 1024], BF16, tag="pt")
                            nc.scalar.activation(pt, st, AF.Exp)
                            nc.tensor.matmul(ota,
                                             vhat[:, kb, ha * 65:(ha + 1) * 65],
                                             pt[:, 0:512],
                                             start=(kb == 0), stop=(kb == KBC - 1))
                            nc.tensor.matmul(otb,
                                             vhat[:, kb, hb * 65:(hb + 1) * 65],
                                             pt[:, 512:1024],
                                             start=(kb == 0), stop=(kb == KBC - 1))
                        for h, ot in ((ha, ota), (hb, otb)):
                            if first:
                                nc.vector.tensor_copy(acc[:, h, qh, :], ot)
                            else:
                                nc.vector.tensor_add(acc[:, h, qh, :],
                                                     acc[:, h, qh, :], ot)

            # softmax divide, batched at the tail.  Denominators live on
            # partition 64 of acc; DVE is partition-lockstep, so DMA-gather
            # them to aligned bases 0 (even heads) / 32 (odd heads), compute
            # 1/den = exp(-ln(den)) on the then-idle scalar engine (the
            # iterative DVE reciprocal is 8 cyc/elem; custom-DVE approx ops
            # don't compile on this walrus), broadcast via K=1 matmuls, and
            # multiply.  Odd heads land in oh2 (partitions 0:64) and are
            # DMA-shifted to ohp[64:128] afterwards.
            def normalize_qh(qh):
                qsl = bass.ts(qh, 512)
                # den rows: 0 = even heads (cols hp*512), 32 = odd heads
                den = pksq.tile([33, 6 * 512], BF16, tag="ksq", name="den")
                for hp in range(6):
                    csl = bass.ts(hp, 512)
                    nc.sync.dma_start(out=den[0:1, csl],
                                      in_=acc[64:65, 2 * hp, qh, :])
                    nc.sync.dma_start(out=den[32:33, csl],
                                      in_=acc[64:65, 2 * hp + 1, qh, :])
                linv = persist.tile([33, 6 * 512], F32, tag="qhat",
                                    name=f"linv_{qh}")
                for bs in (slice(0, 1), slice(32, 33)):
                    nc.scalar.activation(linv[bs, :], den[bs, :], AF.Ln)
                    nc.scalar.activation(linv[bs, :], linv[bs, :], AF.Exp,
                                         scale=-1.0)
                for hp in range(6):
                    csl = bass.ts(hp, 512)
                    for j, h in ((0, 2 * hp), (1, 2 * hp + 1)):
                        bs = slice(32 * j, 32 * j + 1)
                        rbc = pP.tile([128, 512], F32, tag="pP",
                                      name=f"rbc_{h}_{qh}")
                        nc.tensor.matmul(rbc[0:64, :], ones33[bs, :],
                                         linv[bs, csl], start=True, stop=True)
                        dst = (ohp[0:64, hp, qsl] if h % 2 == 0
                               else oh2_box[0][0:64, hp, qsl])
                        nc.vector.tensor_mul(dst, acc[0:64, h, qh, :],
                                             rbc[0:64, :])

            # ---------------- pipeline ----------------
            order = list(range(NCH))
            kv = {}
            kv[0] = proj_chunk(0, True)
            if debug:
                nc.sync.dma_start(
                    out=dbg_qhat[:, :],
                    in_=qhat.rearrange("p a b -> p (a b)"))
                nc.sync.dma_start(
                    out=dbg_khat[:, :],
                    in_=kv[0][0].rearrange("p a b -> p (a b)"))
                nc.sync.dma_start(
                    out=dbg_vhat[:, :],
                    in_=kv[0][1].rearrange("p a b -> p (a b)"))
            kv[1] = proj_chunk(1, False)
            oh2_box = [None]
            # oh2 shares the xT pool slot (free after the last proj chunk)
            for i, c in enumerate(order):
                if i + 2 < NCH:
                    kv[c + 2] = proj_chunk(c + 2, False)
                if i + 2 == NCH:
                    oh2_box[0] = pxT.tile([64, 6, NQ], BF16, tag="xT",
                                          name="oh2")
                attn_chunk(i, *kv[c], first=(i == 0), last=(i == NCH - 1))

            if debug:
                nc.sync.dma_start(
                    out=dbg_acc[:, :],
                    in_=acc.rearrange("p a b c -> p (a b c)"))
            normalize_qh(0)
            normalize_qh(1)
            oh2 = oh2_box[0]
            nc.sync.dma_start(out=ohp[64:128, :, :], in_=oh2[0:64, :, :])
            if debug:
                nc.sync.dma_start(
                    out=dbg_ohp[:, :],
                    in_=ohp.rearrange("p a b -> p (a b)"))

            # ---------------- output projection ----------------
            # Row-tiled halves must accumulate into SEPARATE psum tiles:
            # concurrent row-group matmuls into the same bank crash the HW.
            for mt in range(NQ // 128):
                ypa = pS.tile([128, 1024], F32, tag="pS", name=f"ypa_{mt}")
                ypb = pS.tile([128, 1024], F32, tag="pS", name=f"ypb_{mt}")
                for hp in range(6):
                    for lo, hi, yp in ((0, 64, ypa), (64, 128, ypb)):
                        lhsT = ohp[lo:hi, hp, mt * 128:(mt + 1) * 128]
                        stop = (hp == 5 and lo == 64)
                        nc.tensor.matmul(yp[:, 0:512], lhsT,
                                         wo[lo:hi, hp, 0:512],
                                         start=(hp == 0), stop=stop,
                                         tile_position=(lo, 0))
                        nc.tensor.matmul(yp[:, 512:768], lhsT,
                                         wo[lo:hi, hp, 512:768],
                                         start=(hp == 0), stop=stop,
                                         tile_position=(lo, 0))
                nc.tensor.matmul(ypa[:, 0:512], ones_bf, b_bf[:, 0:512],
                                 start=False, stop=True)
                nc.tensor.matmul(ypa[:, 512:768], ones_bf,
                                 b_bf[:, 512:768], start=False, stop=True)
                ys = pys.tile([128, D], F32, tag="ys")
                nc.vector.tensor_copy(ys, ypa[:, 0:768])
                nc.vector.tensor_add(ys, ys, ypb[:, 0:768])
                nc.sync.dma_start(out=y[mt * 128:(mt + 1) * 128, :], in_=ys)

    _split_multi_waits(nc)
    return nc


_prog_cache = {}
_BF = ml_dtypes.bfloat16


def _make_in_maps(inputs):
    x = np.asarray(inputs["x"], dtype=np.float32)
    w_qkv = np.asarray(inputs["w_qkv"], dtype=np.float32)
    w_out = np.asarray(inputs["w_out"], dtype=np.float32)
    b_out = np.asarray(inputs["b_out"], dtype=np.float32).reshape(1, D)

    wqkvT = w_qkv.T                                  # [768, 2304]
    wq_host = np.ascontiguousarray(
        wqkvT.reshape(6, 128, 3 * INNER).transpose(1, 0, 2).reshape(128, -1)
    ).astype(_BF)
    woT = w_out.T                                    # [768 inner, 768 d]
    wo_host = np.ascontiguousarray(
        woT.reshape(6, 2, 64, D).transpose(1, 2, 0, 3).reshape(128, -1)
    ).astype(_BF)
    b_host = b_out.astype(_BF)
    p = np.arange(128)
    sel = (p[:, None] % 64 == p[None, :] % 64).astype(_BF)

    in_maps = []
    for c in range(NCORES):
        bi, qi = c // 4, c % 4
        xr = np.ascontiguousarray(
            np.roll(x[bi], -qi * NQ, axis=0)).astype(_BF)
        in_maps.append({
            "xb": xr,
            "wqkv": wq_host,
            "wo": wo_host,
            "bout": b_host,
            "selin": sel,
        })
    return in_maps


def kernel(x, w_qkv, w_out, b_out, scale):
    scale = np.asarray(scale, dtype=np.float32)
    inv_scale = tuple(float(1.0 / s) for s in scale)
    nc = _prog_cache.get(inv_scale)
    if nc is None:
        nc = _build_program(inv_scale)
        _prog_cache[inv_scale] = nc

    in_maps = _make_in_maps(
        {"x": x, "w_qkv": w_qkv, "w_out": w_out, "b_out": b_out})

    res = run_bass_kernel_spmd(nc, in_maps, core_ids=list(range(NCORES)))
    out = np.empty((B, N, D), dtype=np.float32)
    for c in range(NCORES):
        bi, qi = c // 4, c % 4
        out[bi, qi * NQ:(qi + 1) * NQ] = res.results[c]["y"]
    return out


# revision 22
# speedup vs baseline: 1.4472x; 1.0814x over previous
"""CosineSimilarityAttention Trainium2 kernel (8 NeuronCores, SPMD).

v2: chunk-pipelined design. Global tokens = 2 batches x 4096; core c handles
batch (c // 4) and query rows (c % 4)*1024 .. +1024. The host rotates each
core's batch so its query tokens are rows 0:1024, and pre-converts x and the
weights to bf16 so the kernel DMA-transposes x straight from DRAM.

Per chunk of 1024 key tokens (4 chunks):
  P(c): xT via DMA-transpose; k (and, on chunk 0, q) projections + the
        head-axis norm  k * (ssq)^-1/4  computed as exp(-0.25*ln(ssq)) so the
        scalar engine stays on one activation table set; v projection into a
        65-stride layout with a ones column (softmax denominator trick).
  A(c): for each head pair hp and query half qh: 8x [K=64 row-tiled score
        matmul pair] -> exp -> AV accumulate in PSUM; then add into an SBUF
        accumulator.  P(c+1) is emitted before A(c) so projections fill
        engine gaps under the exp-bound attention stream.
Tail: softmax denominators -> fast reciprocal -> ones-matmul broadcast ->
      normalize -> output projection (row-tiled pairs) + bias -> DMA out.
"""

import numpy as np
import ml_dtypes

import concourse.bass as bass
import concourse.mybir as mybir
import concourse.tile as tile
from concourse.bass_utils import run_bass_kernel_spmd

F32 = mybir.dt.float32
BF16 = mybir.dt.bfloat16
AF = mybir.ActivationFunctionType

B = 2
N = 4096          # tokens per batch
D = 768           # model dim
H = 12            # heads
DH = 64           # head dim
INNER = H * DH    # 768
NQ = 1024         # query tokens per core
NCORES = 8
CH = 1024         # key-chunk tokens
NCH = N // CH     # 4 chunks
KBC = CH // 128   # 8 key tiles of 128 per chunk
BLK = 512         # projection token block (2 per chunk)


def _split_multi_waits(nc):
    """This container's walrus accepts only ONE sync-wait per instruction.
    Hoist extra waits into standalone EVSEM instructions placed just before."""
    n = 0
    for f in nc.m.functions:
        for bb in f.blocks:
            insts = list(bb.instructions)
            out = []
            for inst in insts:
                si = inst.sync_info
                if si is not None and si.on_wait is not None and len(si.on_wait) > 1:
                    waits = list(si.on_wait)
                    for j, w in enumerate(waits[:-1]):
                        ev = mybir.InstEventSemaphore(
                            name=f"{inst.name}-evw{j}",
                            engine=inst.engine,
                            sync_info=mybir.SyncInfo(on_wait=[w], on_update=[]),
                        )
                        out.append(ev)
                        n += 1
                    si.on_wait = [waits[-1]]
                out.append(inst)
            bb.instructions = out
    return n


def _build_program(inv_scale, debug=False):
    nc = bass.Bass()
    xb = nc.declare_dram_parameter("xb", [N, D], BF16, isOutput=False)
    if debug:
        dbg_qhat = nc.declare_dram_parameter("dbg_qhat", [128, 6 * NQ], BF16,
                                             isOutput=True)
        dbg_khat = nc.declare_dram_parameter("dbg_khat", [128, 6 * CH], BF16,
                                             isOutput=True)
        dbg_vhat = nc.declare_dram_parameter("dbg_vhat", [128, KBC * H * 65],
                                             BF16, isOutput=True)
        dbg_acc = nc.declare_dram_parameter("dbg_acc", [65, H * 2 * 512], BF16,
                                            isOutput=True)
        dbg_ohp = nc.declare_dram_parameter("dbg_ohp", [128, 6 * NQ], BF16,
                                            isOutput=True)
    wqkv = nc.declare_dram_parameter("wqkv", [128, 6 * 3 * INNER], BF16,
                                     isOutput=False)
    wo_in = nc.declare_dram_parameter("wo", [128, 6 * D], BF16, isOutput=False)
    bout = nc.declare_dram_parameter("bout", [1, D], BF16, isOutput=False)
    selin = nc.declare_dram_parameter("selin", [128, 128], BF16, isOutput=False)
    y = nc.declare_dram_parameter("y", [NQ, D], F32, isOutput=True)

    with tile.TileContext(nc) as tc:
        with tc.tile_pool(name="const", bufs=1) as constp, \
             tc.tile_pool(name="persist", bufs=1) as persist, \
             tc.tile_pool(name="pkh", bufs=2) as pkh, \
             tc.tile_pool(name="pvh", bufs=2) as pvh, \
             tc.tile_pool(name="pxT", bufs=1) as pxT, \
             tc.tile_pool(name="pkf", bufs=2) as pkf, \
             tc.tile_pool(name="pksq", bufs=1) as pksq, \
             tc.tile_pool(name="pnrm", bufs=2) as pnrm, \
             tc.tile_pool(name="ppt", bufs=4) as ppt, \
             tc.tile_pool(name="pys", bufs=2) as pys, \
             tc.tile_pool(name="psS", bufs=2, space="PSUM") as pS, \
             tc.tile_pool(name="psO", bufs=2, space="PSUM") as pO, \
             tc.tile_pool(name="psP", bufs=2, space="PSUM") as pP:

            # ---- constants / weights ----
            sel_bf = constp.tile([128, 128], BF16)
            nc.sync.dma_start(out=sel_bf, in_=selin[:, :])
            b_bf = constp.tile([1, D], BF16)
            nc.sync.dma_start(out=b_bf, in_=bout[:, :])
            ones33 = constp.tile([33, 64], BF16)
            nc.vector.memset(ones33, 1.0)
            ones_bf = constp.tile([1, 128], BF16)
            nc.vector.memset(ones_bf, 1.0)
            invs = constp.tile([128, 6], F32)
            for dt in range(6):
                nc.vector.memset(invs[0:64, dt:dt + 1], float(inv_scale[2 * dt]))
                nc.vector.memset(invs[64:128, dt:dt + 1],
                                 float(inv_scale[2 * dt + 1]))
            wq = constp.tile([128, 6, 3 * INNER], BF16)
            nc.sync.dma_start(out=wq, in_=wqkv[:, :].rearrange(
                "p (k c) -> p k c", k=6))
            wo = constp.tile([128, 6, D], BF16)
            nc.sync.dma_start(out=wo, in_=wo_in[:, :].rearrange(
                "p (k c) -> p k c", k=6))

            # ---- persistent activations ----
            qhat = persist.tile([128, 6, NQ], BF16)     # pair layout, 1/scale folded
            acc = persist.tile([65, H, 2, 512], BF16)   # rows 0:64 num, row 64 den
            ohp = persist.tile([128, 6, NQ], BF16)      # normalized out, pair layout

            # ================= projection of one chunk =================
            # Generator: yields between instruction groups so the driver can
            # interleave this chunk's projection into the previous chunk's
            # attention stream (Tile priority follows emission order; bulk
            # emission would let proj bursts starve the exp pipeline).
            def proj_chunk(c, with_q, box):
                xT = pxT.tile([128, 6, CH], BF16, tag="xT",
                              name=f"xT_{c}")
                for dt in range(6):
                    nc.sync.dma_start(
                        out=xT[:, dt, :],
                        in_=xb[c * CH:(c + 1) * CH, dt * 128:(dt + 1) * 128],
                        transpose=True)
                khat = pkh.tile([128, 6, CH], BF16, tag="khat",
                                name=f"khat_{c}")
                vhat = pvh.tile([128, KBC, H * 65], BF16, tag="vhat",
                                name=f"vhat_{c}")
                box[c] = (khat, vhat)
                vones = vhat.rearrange("p t (h c) -> p t h c", c=65)[:, :, :, 64:65]
                nc.vector.memset(vones, 1.0)
                yield

                for blk in range(CH // BLK):
                    bsl = bass.ts(blk, BLK)

                    def qk_side(wbase, is_q):
                        kf = pkf.tile([128, 6, BLK], BF16, tag="kf")
                        ksq = pksq.tile([128, 6, BLK], BF16, tag="ksq")
                        for dt in range(6):
                            kp = pP.tile([128, BLK], F32, tag="pP")
                            for ks in range(6):
                                nc.tensor.matmul(
                                    kp,
                                    wq[:, ks, wbase + dt * 128:
                                       wbase + (dt + 1) * 128],
                                    xT[:, ks, bsl],
                                    start=(ks == 0), stop=(ks == 5))
                            nc.vector.tensor_copy(kf[:, dt, :], kp)
                            nc.vector.tensor_mul(ksq[:, dt, :], kf[:, dt, :],
                                                 kf[:, dt, :])
                            yield
                        sq = pP.tile([128, BLK], F32, tag="pP")
                        for dt in range(6):
                            nc.tensor.matmul(sq, sel_bf, ksq[:, dt, :],
                                             start=(dt == 0), stop=(dt == 5))
                        # rq = ssq^(-1/4) = exp(-0.25 * ln(ssq)); eps dropped
                        # (||k|| ~ 3.5 >> 1e-8).  Ln+Exp share one ACT table set.
                        lnt = pnrm.tile([128, BLK], F32, tag="lnt")
                        nc.scalar.activation(lnt, sq, AF.Ln)
                        rq = pnrm.tile([128, BLK], F32, tag="rq")
                        nc.scalar.activation(rq, lnt, AF.Exp, scale=-0.25)
                        yield
                        for dt in range(6):
                            dst = qhat[:, dt, bsl] if is_q else khat[:, dt, bsl]
                            nc.vector.tensor_mul(dst, kf[:, dt, :], rq)
                            if is_q:
                                nc.vector.tensor_scalar_mul(
                                    dst, dst, invs[:, dt:dt + 1])
                            if dt == 2:
                                yield
                        yield

                    yield from qk_side(INNER, False)
                    if with_q:
                        yield from qk_side(0, True)

                    for tt in range(4):
                        vp1 = pP.tile([128, BLK], F32, tag="pP")
                        vp2 = pP.tile([128, 256], F32, tag="pP")
                        tsl = bass.ds(blk * BLK + tt * 128, 128)
                        for ks in range(6):
                            nc.tensor.matmul(vp1, xT[:, ks, tsl],
                                             wq[:, ks, 2 * INNER:2 * INNER + 512],
                                             start=(ks == 0), stop=(ks == 5))
                        yield
                        for ks in range(6):
                            nc.tensor.matmul(vp2, xT[:, ks, tsl],
                                             wq[:, ks, 2 * INNER + 512:3 * INNER],
                                             start=(ks == 0), stop=(ks == 5))
                        vdst = vhat[:, blk * 4 + tt, :].rearrange(
                            "p (h c) -> p h c", c=65)
                        nc.vector.tensor_copy(
                            vdst[:, 0:8, 0:64],
                            vp1.rearrange("p (h c) -> p h c", c=64))
                        nc.vector.tensor_copy(
                            vdst[:, 8:12, 0:64],
                            vp2.rearrange("p (h c) -> p h c", c=64))
                        yield

            # ================= attention over one chunk =================
            # `gen` is the next chunk's projection generator; one step is
            # emitted every other kb iteration so proj work lands between
            # attention instructions in scheduler priority.
            def attn_chunk(ci, khat, vhat, first, gen):
                tick = 0
                for hp in range(6):
                    ha, hb = 2 * hp, 2 * hp + 1
                    for qh in range(2):
                        qsl = bass.ts(qh, 512)
                        ota = pO.tile([65, 512], F32, tag="ot",
                                      name=f"ota_{ci}_{hp}_{qh}")
                        otb = pO.tile([65, 512], F32, tag="ot",
                                      name=f"otb_{ci}_{hp}_{qh}")
                        for kb in range(KBC):
                            ksl = bass.ts(kb, 128)
                            st = pS.tile([128, 1024], F32, tag="pS")
                            nc.tensor.matmul(st[:, 0:512],
                                             khat[0:64, hp, ksl],
                                             qhat[0:64, hp, qsl],
                                             start=True, stop=True,
                                             tile_position=(0, 0))
                            nc.tensor.matmul(st[:, 512:1024],
                                             khat[64:128, hp, ksl],
                                             qhat[64:128, hp, qsl],
                                             start=True, stop=True,
                                             tile_position=(64, 0))
                            pt = ppt.tile([128, 1024], BF16, tag="pt")
                            nc.scalar.activation(pt, st, AF.Exp)
                            nc.tensor.matmul(ota,
                                             vhat[:, kb, ha * 65:(ha + 1) * 65],
                                             pt[:, 0:512],
                                             start=(kb == 0), stop=(kb == KBC - 1))
                            nc.tensor.matmul(otb,
                                             vhat[:, kb, hb * 65:(hb + 1) * 65],
                                             pt[:, 512:1024],
                                             start=(kb == 0), stop=(kb == KBC - 1))
                            tick += 1
                            if gen is not None and tick % 2 == 0:
                                next(gen, None)
                        for h, ot in ((ha, ota), (hb, otb)):
                            if first:
                                nc.vector.tensor_copy(acc[:, h, qh, :], ot)
                            else:
                                nc.vector.tensor_add(acc[:, h, qh, :],
                                                     acc[:, h, qh, :], ot)
                while gen is not None and next(gen, StopIteration) is not StopIteration:
                    pass

            # softmax divide, batched at the tail.  Denominators live on
            # partition 64 of acc; DVE is partition-lockstep, so DMA-gather
            # them to aligned bases 0 (even heads) / 32 (odd heads), compute
            # 1/den = exp(-ln(den)) on the then-idle scalar engine (the
            # iterative DVE reciprocal is 8 cyc/elem; custom-DVE approx ops
            # don't compile on this walrus), broadcast via K=1 matmuls, and
            # multiply.  Odd heads land in oh2 (partitions 0:64) and are
            # DMA-shifted to ohp[64:128] afterwards.
            def normalize_all():
                # den rows: 0 = even heads, 32 = odd heads; cols (qh, hp)*512
                den = pksq.tile([33, 12 * 512], BF16, tag="ksq", name="den")
                for qh in range(2):
                    for hp in range(6):
                        csl = bass.ds((qh * 6 + hp) * 512, 512)
                        nc.sync.dma_start(out=den[0:1, csl],
                                          in_=acc[64:65, 2 * hp, qh, :])
                        nc.sync.dma_start(out=den[32:33, csl],
                                          in_=acc[64:65, 2 * hp + 1, qh, :])
                linv = persist.tile([33, 12 * 512], BF16, tag="qhat",
                                    name="linv")
                lntd = pxT.tile([33, 6 * 512], F32, tag="xT", name="lntd")
                for qh in range(2):
                    for bs in (slice(0, 1), slice(32, 33)):
                        nc.scalar.activation(
                            lntd[bs, :],
                            den[bs, qh * 3072:(qh + 1) * 3072], AF.Ln)
                        nc.scalar.activation(
                            linv[bs, qh * 3072:(qh + 1) * 3072],
                            lntd[bs, :], AF.Exp, scale=-1.0)
                oh2 = pxT.tile([64, 6, NQ], BF16, tag="xT", name="oh2")
                oh2_box[0] = oh2
                # alternate rbc between two psum pools so the K=1 broadcast
                # matmuls pipeline ahead of the DVE multiplies.
                for qh in range(2):
                    qsl = bass.ts(qh, 512)
                    for hp in range(6):
                        csl = bass.ds((qh * 6 + hp) * 512, 512)
                        for j, h in ((0, 2 * hp), (1, 2 * hp + 1)):
                            bs = slice(32 * j, 32 * j + 1)
                            if (hp + j) % 2 == 0:
                                rbc = pP.tile([128, 512], F32, tag="pP",
                                              name=f"rbc_{h}_{qh}")
                            else:
                                rbc = pS.tile([128, 1024], F32, tag="pS",
                                              name=f"rbc_{h}_{qh}")
                            nc.tensor.matmul(rbc[0:64, 0:512], ones33[bs, :],
                                             linv[bs, csl],
                                             start=True, stop=True)
                            dst = (ohp[0:64, hp, qsl] if h % 2 == 0
                                   else oh2[0:64, hp, qsl])
                            nc.vector.tensor_mul(dst, acc[0:64, h, qh, :],
                                                 rbc[0:64, 0:512])

            # ---------------- pipeline ----------------
            box = {}
            oh2_box = [None]
            g0 = proj_chunk(0, True, box)
            for _ in g0:
                pass
            if debug:
                nc.sync.dma_start(
                    out=dbg_qhat[:, :],
                    in_=qhat.rearrange("p a b -> p (a b)"))
                nc.sync.dma_start(
                    out=dbg_khat[:, :],
                    in_=box[0][0].rearrange("p a b -> p (a b)"))
                nc.sync.dma_start(
                    out=dbg_vhat[:, :],
                    in_=box[0][1].rearrange("p a b -> p (a b)"))
            gens = {c: proj_chunk(c, False, box) for c in range(1, NCH)}
            next(gens[1], None)     # chunk 1 transposes issue early
            for i in range(NCH):
                attn_chunk(i, *box[i], first=(i == 0), gen=gens.get(i + 1))

            if debug:
                nc.sync.dma_start(
                    out=dbg_acc[:, :],
                    in_=acc.rearrange("p a b c -> p (a b c)"))
            normalize_all()
            nc.sync.dma_start(out=ohp[64:128, :, :], in_=oh2_box[0][0:64, :, :])
            if debug:
                nc.sync.dma_start(
                    out=dbg_ohp[:, :],
                    in_=ohp.rearrange("p a b -> p (a b)"))

            # ---------------- output projection ----------------
            # Row-tiled halves must accumulate into SEPARATE psum tiles:
            # concurrent row-group matmuls into the same bank crash the HW.
            for mt in range(NQ // 128):
                ypa = pS.tile([128, 1024], F32, tag="pS", name=f"ypa_{mt}")
                ypb = pS.tile([128, 1024], F32, tag="pS", name=f"ypb_{mt}")
                for hp in range(6):
                    for lo, hi, yp in ((0, 64, ypa), (64, 128, ypb)):
                        lhsT = ohp[lo:hi, hp, mt * 128:(mt + 1) * 128]
                        stop = (hp == 5 and lo == 64)
                        nc.tensor.matmul(yp[:, 0:512], lhsT,
                                         wo[lo:hi, hp, 0:512],
                                         start=(hp == 0), stop=stop,
                                         tile_position=(lo, 0))
                        nc.tensor.matmul(yp[:, 512:768], lhsT,
                                         wo[lo:hi, hp, 512:768],
                                         start=(hp == 0), stop=stop,
                                         tile_position=(lo, 0))
                nc.tensor.matmul(ypa[:, 0:512], ones_bf, b_bf[:, 0:512],
                                 start=False, stop=True)
                nc.tensor.matmul(ypa[:, 512:768], ones_bf,
                                 b_bf[:, 512:768], start=False, stop=True)
                ys = pys.tile([128, D], F32, tag="ys")
                nc.vector.tensor_copy(ys, ypa[:, 0:768])
                nc.vector.tensor_add(ys, ys, ypb[:, 0:768])
                nc.sync.dma_start(out=y[mt * 128:(mt + 1) * 128, :], in_=ys)

    _split_multi_waits(nc)
    return nc


_prog_cache = {}
_BF = ml_dtypes.bfloat16


def _make_in_maps(inputs):
    x = np.asarray(inputs["x"], dtype=np.float32)
    w_qkv = np.asarray(inputs["w_qkv"], dtype=np.float32)
    w_out = np.asarray(inputs["w_out"], dtype=np.float32)
    b_out = np.asarray(inputs["b_out"], dtype=np.float32).reshape(1, D)

    wqkvT = w_qkv.T                                  # [768, 2304]
    wq_host = np.ascontiguousarray(
        wqkvT.reshape(6, 128, 3 * INNER).transpose(1, 0, 2).reshape(128, -1)
    ).astype(_BF)
    woT = w_out.T                                    # [768 inner, 768 d]
    wo_host = np.ascontiguousarray(
        woT.reshape(6, 2, 64, D).transpose(1, 2, 0, 3).reshape(128, -1)
    ).astype(_BF)
    b_host = b_out.astype(_BF)
    p = np.arange(128)
    sel = (p[:, None] % 64 == p[None, :] % 64).astype(_BF)

    in_maps = []
    for c in range(NCORES):
        bi, qi = c // 4, c % 4
        xr = np.ascontiguousarray(
            np.roll(x[bi], -qi * NQ, axis=0)).astype(_BF)
        in_maps.append({
            "xb": xr,
            "wqkv": wq_host,
            "wo": wo_host,
            "bout": b_host,
            "selin": sel,
        })
    return in_maps


def kernel(x, w_qkv, w_out, b_out, scale):
    scale = np.asarray(scale, dtype=np.float32)
    inv_scale = tuple(float(1.0 / s) for s in scale)
    nc = _prog_cache.get(inv_scale)
    if nc is None:
        nc = _build_program(inv_scale)
        _prog_cache[inv_scale] = nc

    in_maps = _make_in_maps(
        {"x": x, "w_qkv": w_qkv, "w_out": w_out, "b_out": b_out})

    res = run_bass_kernel_spmd(nc, in_maps, core_ids=list(range(NCORES)))
    out = np.empty((B, N, D), dtype=np.float32)
    for c in range(NCORES):
        bi, qi = c // 4, c % 4
        out[bi, qi * NQ:(qi + 1) * NQ] = res.results[c]["y"]
    return out


# revision 26
# speedup vs baseline: 1.4836x; 1.0251x over previous
"""CosineSimilarityAttention Trainium2 kernel (8 NeuronCores, SPMD).

v2: chunk-pipelined design. Global tokens = 2 batches x 4096; core c handles
batch (c // 4) and query rows (c % 4)*1024 .. +1024. The host rotates each
core's batch so its query tokens are rows 0:1024, and pre-converts x and the
weights to bf16 so the kernel DMA-transposes x straight from DRAM.

Per chunk of 1024 key tokens (4 chunks):
  P(c): xT via DMA-transpose; k (and, on chunk 0, q) projections + the
        head-axis norm  k * (ssq)^-1/4  computed as exp(-0.25*ln(ssq)) so the
        scalar engine stays on one activation table set; v projection into a
        65-stride layout with a ones column (softmax denominator trick).
  A(c): for each head pair hp and query half qh: 8x [K=64 row-tiled score
        matmul pair] -> exp -> AV accumulate in PSUM; then add into an SBUF
        accumulator.  P(c+1) is emitted before A(c) so projections fill
        engine gaps under the exp-bound attention stream.
Tail: softmax denominators -> fast reciprocal -> ones-matmul broadcast ->
      normalize -> output projection (row-tiled pairs) + bias -> DMA out.
"""

import numpy as np
import ml_dtypes

import concourse.bass as bass
import concourse.mybir as mybir
import concourse.tile as tile
from concourse.bass_utils import run_bass_kernel_spmd

F32 = mybir.dt.float32
BF16 = mybir.dt.bfloat16
AF = mybir.ActivationFunctionType

B = 2
N = 4096          # tokens per batch
D = 768           # model dim
H = 12            # heads
DH = 64           # head dim
INNER = H * DH    # 768
NQ = 1024         # query tokens per core
NCORES = 8
CH = 1024         # key-chunk tokens
NCH = N // CH     # 4 chunks
KBC = CH // 128   # 8 key tiles of 128 per chunk
BLK = 512         # projection token block (2 per chunk)


def _split_multi_waits(nc):
    """This container's walrus accepts only ONE sync-wait per instruction.
    Hoist extra waits into standalone EVSEM instructions placed just before."""
    n = 0
    for f in nc.m.functions:
        for bb in f.blocks:
            insts = list(bb.instructions)
            out = []
            for inst in insts:
                si = inst.sync_info
                if si is not None and si.on_wait is not None and len(si.on_wait) > 1:
                    waits = list(si.on_wait)
                    for j, w in enumerate(waits[:-1]):
                        ev = mybir.InstEventSemaphore(
                            name=f"{inst.name}-evw{j}",
                            engine=inst.engine,
                            sync_info=mybir.SyncInfo(on_wait=[w], on_update=[]),
                        )
                        out.append(ev)
                        n += 1
                    si.on_wait = [waits[-1]]
                out.append(inst)
            bb.instructions = out
    return n


def _build_program(inv_scale, debug=False):
    nc = bass.Bass()
    xb = nc.declare_dram_parameter("xb", [N, D], BF16, isOutput=False)
    if debug:
        dbg_qhat = nc.declare_dram_parameter("dbg_qhat", [128, 6 * NQ], BF16,
                                             isOutput=True)
        dbg_khat = nc.declare_dram_parameter("dbg_khat", [128, 6 * CH], BF16,
                                             isOutput=True)
        dbg_vhat = nc.declare_dram_parameter("dbg_vhat", [128, KBC * H * 65],
                                             BF16, isOutput=True)
        dbg_acc = nc.declare_dram_parameter("dbg_acc", [65, H * 2 * 512], BF16,
                                            isOutput=True)
        dbg_ohp = nc.declare_dram_parameter("dbg_ohp", [128, 6 * NQ], BF16,
                                            isOutput=True)
    wqkv = nc.declare_dram_parameter("wqkv", [128, 6 * 3 * INNER], BF16,
                                     isOutput=False)
    wo_in = nc.declare_dram_parameter("wo", [128, 6 * D], BF16, isOutput=False)
    bout = nc.declare_dram_parameter("bout", [1, D], BF16, isOutput=False)
    selin = nc.declare_dram_parameter("selin", [128, 128], BF16, isOutput=False)
    y = nc.declare_dram_parameter("y", [NQ, D], F32, isOutput=True)

    with tile.TileContext(nc) as tc:
        with tc.tile_pool(name="const", bufs=1) as constp, \
             tc.tile_pool(name="persist", bufs=1) as persist, \
             tc.tile_pool(name="pkh", bufs=2) as pkh, \
             tc.tile_pool(name="pvh", bufs=2) as pvh, \
             tc.tile_pool(name="pxT", bufs=1) as pxT, \
             tc.tile_pool(name="pkf", bufs=2) as pkf, \
             tc.tile_pool(name="pksq", bufs=1) as pksq, \
             tc.tile_pool(name="pnrm", bufs=2) as pnrm, \
             tc.tile_pool(name="ppt", bufs=4) as ppt, \
             tc.tile_pool(name="pys", bufs=2) as pys, \
             tc.tile_pool(name="psS", bufs=2, space="PSUM") as pS, \
             tc.tile_pool(name="psO", bufs=2, space="PSUM") as pO, \
             tc.tile_pool(name="psP", bufs=2, space="PSUM") as pP:

            # ---- constants / weights ----
            sel_bf = constp.tile([128, 128], BF16)
            nc.sync.dma_start(out=sel_bf, in_=selin[:, :])
            b_bf = constp.tile([1, D], BF16)
            nc.sync.dma_start(out=b_bf, in_=bout[:, :])
            ones33 = constp.tile([33, 64], BF16)
            nc.vector.memset(ones33, 1.0)
            ones_bf = constp.tile([1, 128], BF16)
            nc.vector.memset(ones_bf, 1.0)
            invs = constp.tile([128, 6], F32)
            for dt in range(6):
                nc.vector.memset(invs[0:64, dt:dt + 1], float(inv_scale[2 * dt]))
                nc.vector.memset(invs[64:128, dt:dt + 1],
                                 float(inv_scale[2 * dt + 1]))
            wq = constp.tile([128, 6, 3 * INNER], BF16)
            nc.sync.dma_start(out=wq, in_=wqkv[:, :].rearrange(
                "p (k c) -> p k c", k=6))
            wo = constp.tile([128, 6, D], BF16)
            nc.sync.dma_start(out=wo, in_=wo_in[:, :].rearrange(
                "p (k c) -> p k c", k=6))

            # ---- persistent activations ----
            qhat = persist.tile([128, 6, NQ], BF16)     # pair layout, 1/scale folded
            acc = persist.tile([65, H, 2, 512], BF16)   # rows 0:64 num, row 64 den
            ohp = persist.tile([128, 6, NQ], BF16)      # normalized out, pair layout

            # ================= projection of one chunk =================
            # Generator: yields between instruction groups so the driver can
            # interleave this chunk's projection into the previous chunk's
            # attention stream (Tile priority follows emission order; bulk
            # emission would let proj bursts starve the exp pipeline).
            def proj_chunk(c, with_q, box):
                xT = pxT.tile([128, 6, CH], BF16, tag="xT",
                              name=f"xT_{c}")
                for dt in range(6):
                    nc.sync.dma_start(
                        out=xT[:, dt, :],
                        in_=xb[c * CH:(c + 1) * CH, dt * 128:(dt + 1) * 128],
                        transpose=True)
                khat = pkh.tile([128, 6, CH], BF16, tag="khat",
                                name=f"khat_{c}")
                vhat = pvh.tile([128, KBC, H * 65], BF16, tag="vhat",
                                name=f"vhat_{c}")
                box[c] = (khat, vhat)
                vones = vhat.rearrange("p t (h c) -> p t h c", c=65)[:, :, :, 64:65]
                nc.vector.memset(vones, 1.0)
                yield

                for blk in range(CH // BLK):
                    bsl = bass.ts(blk, BLK)

                    def qk_side(wbase, is_q):
                        kf = pkf.tile([128, 6, BLK], BF16, tag="kf")
                        ksq = pksq.tile([128, 6, BLK], BF16, tag="ksq")
                        for dt in range(6):
                            kp = pP.tile([128, BLK], F32, tag="pP")
                            for ks in range(6):
                                nc.tensor.matmul(
                                    kp,
                                    wq[:, ks, wbase + dt * 128:
                                       wbase + (dt + 1) * 128],
                                    xT[:, ks, bsl],
                                    start=(ks == 0), stop=(ks == 5))
                            nc.vector.tensor_copy(kf[:, dt, :], kp)
                            nc.vector.tensor_mul(ksq[:, dt, :], kf[:, dt, :],
                                                 kf[:, dt, :])
                            yield
                        sq = pP.tile([128, BLK], F32, tag="pP")
                        for dt in range(6):
                            nc.tensor.matmul(sq, sel_bf, ksq[:, dt, :],
                                             start=(dt == 0), stop=(dt == 5))
                        # rq = ssq^(-1/4) = exp(-0.25 * ln(ssq)); eps dropped
                        # (||k|| ~ 3.5 >> 1e-8).  Ln+Exp share one ACT table set.
                        lnt = pnrm.tile([128, BLK], F32, tag="lnt")
                        nc.scalar.activation(lnt, sq, AF.Ln)
                        rq = pnrm.tile([128, BLK], F32, tag="rq")
                        nc.scalar.activation(rq, lnt, AF.Exp, scale=-0.25)
                        yield
                        for dt in range(6):
                            dst = qhat[:, dt, bsl] if is_q else khat[:, dt, bsl]
                            nc.vector.tensor_mul(dst, kf[:, dt, :], rq)
                            if is_q:
                                nc.vector.tensor_scalar_mul(
                                    dst, dst, invs[:, dt:dt + 1])
                            if dt == 2:
                                yield
                        yield

                    yield from qk_side(INNER, False)
                    if with_q:
                        yield from qk_side(0, True)

                    for tt in range(4):
                        vp1 = pP.tile([128, BLK], F32, tag="pP")
                        vp2 = pP.tile([128, 256], F32, tag="pP")
                        tsl = bass.ds(blk * BLK + tt * 128, 128)
                        for ks in range(6):
                            nc.tensor.matmul(vp1, xT[:, ks, tsl],
                                             wq[:, ks, 2 * INNER:2 * INNER + 512],
                                             start=(ks == 0), stop=(ks == 5))
                        yield
                        for ks in range(6):
                            nc.tensor.matmul(vp2, xT[:, ks, tsl],
                                             wq[:, ks, 2 * INNER + 512:3 * INNER],
                                             start=(ks == 0), stop=(ks == 5))
                        vdst = vhat[:, blk * 4 + tt, :].rearrange(
                            "p (h c) -> p h c", c=65)
                        nc.vector.tensor_copy(
                            vdst[:, 0:8, 0:64],
                            vp1.rearrange("p (h c) -> p h c", c=64))
                        nc.vector.tensor_copy(
                            vdst[:, 8:12, 0:64],
                            vp2.rearrange("p (h c) -> p h c", c=64))
                        yield

            # ============ attention for one (chunk, head pair, qh) ============
            # `pump` emits one step of the next chunk's projection generator
            # between kb iterations so proj work lands interleaved in
            # scheduler priority rather than as a starving burst.
            def attn_hq(ci, khat, vhat, hp, qh, first, pump):
                ha, hb = 2 * hp, 2 * hp + 1
                qsl = bass.ts(qh, 512)
                ota = pO.tile([65, 512], F32, tag="ot",
                              name=f"ota_{ci}_{hp}_{qh}")
                otb = pO.tile([65, 512], F32, tag="ot",
                              name=f"otb_{ci}_{hp}_{qh}")
                for kb in range(KBC):
                    ksl = bass.ts(kb, 128)
                    st = pS.tile([128, 1024], F32, tag="pS")
                    nc.tensor.matmul(st[:, 0:512],
                                     khat[0:64, hp, ksl],
                                     qhat[0:64, hp, qsl],
                                     start=True, stop=True,
                                     tile_position=(0, 0))
                    nc.tensor.matmul(st[:, 512:1024],
                                     khat[64:128, hp, ksl],
                                     qhat[64:128, hp, qsl],
                                     start=True, stop=True,
                                     tile_position=(64, 0))
                    pt = ppt.tile([128, 1024], BF16, tag="pt")
                    nc.scalar.activation(pt, st, AF.Exp)
                    nc.tensor.matmul(ota,
                                     vhat[:, kb, ha * 65:(ha + 1) * 65],
                                     pt[:, 0:512],
                                     start=(kb == 0), stop=(kb == KBC - 1))
                    nc.tensor.matmul(otb,
                                     vhat[:, kb, hb * 65:(hb + 1) * 65],
                                     pt[:, 512:1024],
                                     start=(kb == 0), stop=(kb == KBC - 1))
                    if pump is not None and kb % 2 == 1:
                        pump()
                for h, ot in ((ha, ota), (hb, otb)):
                    if first:
                        nc.vector.tensor_copy(acc[:, h, qh, :], ot)
                    else:
                        nc.vector.tensor_add(acc[:, h, qh, :],
                                             acc[:, h, qh, :], ot)

            # softmax divide, batched at the tail.  Denominators live on
            # partition 64 of acc; DVE is partition-lockstep, so DMA-gather
            # them to aligned bases 0 (even heads) / 32 (odd heads), compute
            # 1/den = exp(-ln(den)) on the then-idle scalar engine (the
            # iterative DVE reciprocal is 8 cyc/elem; custom-DVE approx ops
            # don't compile on this walrus), broadcast via K=1 matmuls, and
            # multiply.  Odd heads land in oh2 (partitions 0:64) and are
            # DMA-shifted to ohp[64:128] afterwards.
            # softmax divide for one query half.  Denominators (partition 64
            # of acc) DMA-gather to aligned bases 0 / 32; 1/den = exp(-ln d)
            # on the scalar engine; K=1 ones-matmul broadcast; multiply.
            # Odd heads land in oh2 (partitions 0:64), DMA-shifted to
            # ohp[64:128] after.  Staging reuses slots that are free by the
            # last chunk: den/linv in the ksq slot chain, lntd in xT's,
            # oh2 in a khat slot.
            def normalize_q(qh):
                qsl = bass.ts(qh, 512)
                den = pksq.tile([33, 6 * 512], BF16, tag="ksq",
                                name=f"den_{qh}")
                for hp in range(6):
                    csl = bass.ts(hp, 512)
                    nc.sync.dma_start(out=den[0:1, csl],
                                      in_=acc[64:65, 2 * hp, qh, :])
                    nc.sync.dma_start(out=den[32:33, csl],
                                      in_=acc[64:65, 2 * hp + 1, qh, :])
                lntd = pxT.tile([33, 6 * 512], F32, tag="xT",
                                name=f"lntd_{qh}")
                linv = pksq.tile([33, 6 * 512], BF16, tag="ksq",
                                 name=f"linv_{qh}")
                for bs in (slice(0, 1), slice(32, 33)):
                    nc.scalar.activation(lntd[bs, :], den[bs, :], AF.Ln)
                    nc.scalar.activation(linv[bs, :], lntd[bs, :], AF.Exp,
                                         scale=-1.0)
                for hp in range(6):
                    csl = bass.ts(hp, 512)
                    for j, h in ((0, 2 * hp), (1, 2 * hp + 1)):
                        bs = slice(32 * j, 32 * j + 1)
                        rbc = pP.tile([128, 512], F32, tag="pP",
                                      name=f"rbc_{h}_{qh}")
                        nc.tensor.matmul(rbc[0:64, :], ones33[bs, :],
                                         linv[bs, csl],
                                         start=True, stop=True)
                        dst = (ohp[0:64, hp, qsl] if h % 2 == 0
                               else oh2_box[0][0:64, hp, qsl])
                        nc.vector.tensor_mul(dst, acc[0:64, h, qh, :],
                                             rbc[0:64, :])
                nc.sync.dma_start(out=ohp[64:128, :, qsl],
                                  in_=oh2_box[0][0:64, :, qsl])

            # output projection for one query half (mt = 4 blocks of 128).
            # Column-split passes keep each accumulator in a 1-bank pP tile
            # so the last chunk's st rotation is untouched; row-tiled halves
            # accumulate into separate tiles (same-bank concurrency is fatal).
            def y_half(qh):
                for mt in range(qh * 4, qh * 4 + 4):
                    ys = pys.tile([128, D], F32, tag="ys")
                    for c0, cn in ((0, 512), (512, 256)):
                        ypa = pP.tile([128, 512], F32, tag="pP",
                                      name=f"ypa_{mt}_{c0}")
                        ypb = pP.tile([128, 512], F32, tag="pP",
                                      name=f"ypb_{mt}_{c0}")
                        for hp in range(6):
                            for lo, hi, yp in ((0, 64, ypa), (64, 128, ypb)):
                                lhsT = ohp[lo:hi, hp, mt * 128:(mt + 1) * 128]
                                nc.tensor.matmul(
                                    yp[:, 0:cn], lhsT,
                                    wo[lo:hi, hp, c0:c0 + cn],
                                    start=(hp == 0),
                                    stop=(hp == 5 and lo == 64),
                                    tile_position=(lo, 0))
                        nc.tensor.matmul(ypa[:, 0:cn], ones_bf,
                                         b_bf[:, c0:c0 + cn],
                                         start=False, stop=True)
                        nc.vector.tensor_copy(ys[:, c0:c0 + cn], ypa[:, 0:cn])
                        nc.vector.tensor_add(ys[:, c0:c0 + cn],
                                             ys[:, c0:c0 + cn], ypb[:, 0:cn])
                    nc.sync.dma_start(out=y[mt * 128:(mt + 1) * 128, :], in_=ys)

            # ---------------- pipeline ----------------
            box = {}
            oh2_box = [None]
            g0 = proj_chunk(0, True, box)
            for _ in g0:
                pass
            if debug:
                nc.sync.dma_start(
                    out=dbg_qhat[:, :],
                    in_=qhat.rearrange("p a b -> p (a b)"))
                nc.sync.dma_start(
                    out=dbg_khat[:, :],
                    in_=box[0][0].rearrange("p a b -> p (a b)"))
                nc.sync.dma_start(
                    out=dbg_vhat[:, :],
                    in_=box[0][1].rearrange("p a b -> p (a b)"))
            gens = {c: proj_chunk(c, False, box) for c in range(1, NCH)}
            next(gens[1], None)     # chunk 1 transposes issue early

            def make_pump(g):
                return lambda: next(g, None)

            for i in range(NCH - 1):
                g = gens.get(i + 1)
                pump = make_pump(g) if g is not None else None
                for hp in range(6):
                    for qh in range(2):
                        attn_hq(i, *box[i], hp, qh, first=(i == 0), pump=pump)
                while g is not None and \
                        next(g, StopIteration) is not StopIteration:
                    pass

            # Last chunk runs qh-outer so normalize + output projection of
            # query half 0 overlap the half-1 attention stream.
            oh2_box[0] = pkh.tile([64, 6, NQ], BF16, tag="khat", name="oh2")
            li = NCH - 1
            for qh in range(2):
                for hp in range(6):
                    attn_hq(li, *box[li], hp, qh, first=False, pump=None)
                normalize_q(qh)
                y_half(qh)

            if debug:
                nc.sync.dma_start(
                    out=dbg_acc[:, :],
                    in_=acc.rearrange("p a b c -> p (a b c)"))
                nc.sync.dma_start(
                    out=dbg_ohp[:, :],
                    in_=ohp.rearrange("p a b -> p (a b)"))

    _split_multi_waits(nc)
    return nc


_prog_cache = {}
_BF = ml_dtypes.bfloat16


def _make_in_maps(inputs):
    x = np.asarray(inputs["x"], dtype=np.float32)
    w_qkv = np.asarray(inputs["w_qkv"], dtype=np.float32)
    w_out = np.asarray(inputs["w_out"], dtype=np.float32)
    b_out = np.asarray(inputs["b_out"], dtype=np.float32).reshape(1, D)

    wqkvT = w_qkv.T                                  # [768, 2304]
    wq_host = np.ascontiguousarray(
        wqkvT.reshape(6, 128, 3 * INNER).transpose(1, 0, 2).reshape(128, -1)
    ).astype(_BF)
    woT = w_out.T                                    # [768 inner, 768 d]
    wo_host = np.ascontiguousarray(
        woT.reshape(6, 2, 64, D).transpose(1, 2, 0, 3).reshape(128, -1)
    ).astype(_BF)
    b_host = b_out.astype(_BF)
    p = np.arange(128)
    sel = (p[:, None] % 64 == p[None, :] % 64).astype(_BF)

    in_maps = []
    for c in range(NCORES):
        bi, qi = c // 4, c % 4
        xr = np.ascontiguousarray(
            np.roll(x[bi], -qi * NQ, axis=0)).astype(_BF)
        in_maps.append({
            "xb": xr,
            "wqkv": wq_host,
            "wo": wo_host,
            "bout": b_host,
            "selin": sel,
        })
    return in_maps


def kernel(x, w_qkv, w_out, b_out, scale):
    scale = np.asarray(scale, dtype=np.float32)
    inv_scale = tuple(float(1.0 / s) for s in scale)
    nc = _prog_cache.get(inv_scale)
    if nc is None:
        nc = _build_program(inv_scale)
        _prog_cache[inv_scale] = nc

    in_maps = _make_in_maps(
        {"x": x, "w_qkv": w_qkv, "w_out": w_out, "b_out": b_out})

    res = run_bass_kernel_spmd(nc, in_maps, core_ids=list(range(NCORES)))
    out = np.empty((B, N, D), dtype=np.float32)
    for c in range(NCORES):
        bi, qi = c // 4, c % 4
        out[bi, qi * NQ:(qi + 1) * NQ] = res.results[c]["y"]
    return out
